# revision 1
# baseline (speedup 1.0000x reference)
"""AttnReweight kernel for Trainium2 (8 NeuronCores, SPMD data parallel).

Semantics (matching the reference):
    c = max(attn); a = exp(attn - c)
    pj[b,s,h,w,k] = sum_t sims[b,hj,wj,t] * (sinds[b,hj,wj,t] == sinds[b,h,w,s])
                    where (hj,wj) = clamped 3x3 neighbor k of (h,w)
    m = a[b,d,h,w,k] * pj[b,s,h,w,k]
    out[b,d,s,h,w,k] = m / (1e-10 + sum_k m)

Sharding: core = b*4 + q handles image b, rows [48q, 48q+48), all heads.

On-chip layout (per core): 96 partitions = (w-half 2, row 48); free dim
= (slot s, offset k, w 96) with w innermost, so every hot vector op is
a dense step-1 16-bit op -> 2x DVE packing.  Both s and k live in the
free dim, so the denominator reduction (tree over k) and the rec
broadcast along k are pure within-partition DVE ops: the per-head phase
is a single-engine chain with no PE/PSUM/DRAM round-trips and no
cross-partition DMA at all (the DMA-queue latency that bounded the
previous k-on-partitions design is gone).

Precision: fp16 for ids/sims/pj (sims pre-scaled x4096 to stay normal),
exp pre-scaled by e^10 (cancels in normalization; eps scaled to match),
m/out in bf16 (range), den tree k-pairs in bf16 then f32, reciprocal
f32 on DVE, final rec cast on the idle Scalar engine.  Host does the
final transpose + f32 cast (pure unshard/layout work).
"""

import numpy as np
import ml_dtypes

B, HD, H, W, K, NSP = 2, 8, 192, 192, 9, 9
NCORES = 8
ROWS = 48
WSEG = 96              # interior w per half
WSPAN = 98             # with 1-pixel halo both sides
P = 2 * ROWS           # 96 partitions: p = half*48 + row
FI = K * WSEG          # 864 (k, w) free elements
FS = NSP * K * WSEG    # 7776 (s, k, w) free elements
EPS = 1e-10
EPS_DEV = EPS * 22026.4657948 * 4096.0  # match exp & sims rescales
OFFS = [(dh, dw) for dh in (-1, 0, 1) for dw in (-1, 0, 1)]
BF = ml_dtypes.bfloat16

_compiled = None


def _build():
    from contextlib import ExitStack

    import concourse.bacc as bacc
    import concourse.tile as tile
    from concourse import mybir

    f32 = mybir.dt.float32
    bf16 = mybir.dt.bfloat16
    f16 = mybir.dt.float16
    Alu = mybir.AluOpType
    Act = mybir.ActivationFunctionType

    nc = bacc.Bacc(
        "TRN2",
        target_bir_lowering=False,
        debug=False,
        enable_asserts=True,
        num_devices=NCORES,
    )

    si_d = nc.dram_tensor("si2", [P, NSP * WSEG], f16, kind="ExternalInput").ap()
    sj_d = nc.dram_tensor("sj2", [3, P, K * WSPAN], f16, kind="ExternalInput").ap()
    wj_d = nc.dram_tensor("wj2", [3, P, K * WSPAN], f16, kind="ExternalInput").ap()
    a_d = nc.dram_tensor("a2", [HD, P, FI], f32, kind="ExternalInput").ap()
    negc_d = nc.dram_tensor("negc", [128, 1], f32, kind="ExternalInput").ap()
    eps_d = nc.dram_tensor("epsv", [128, 1], f32, kind="ExternalInput").ap()
    out_d = nc.dram_tensor("out", [HD, P, FS], bf16, kind="ExternalOutput").ap()

    with tile.TileContext(nc) as tc, ExitStack() as ctx:
        const = ctx.enter_context(tc.tile_pool(name="const", bufs=1))
        work = ctx.enter_context(tc.tile_pool(name="work", bufs=2))
        outp = ctx.enter_context(tc.tile_pool(name="outp", bufs=2))

        negc_t = const.tile([128, 1], f32)
        eps_t = const.tile([128, 1], f32)
        si_t = const.tile([P, NSP * WSEG], f16)
        sj_t = [const.tile([P, K * WSPAN], f16, name=f"sj{i}") for i in range(3)]
        wj_t = [const.tile([P, K * WSPAN], f16, name=f"wj{i}") for i in range(3)]
        nc.sync.dma_start(negc_t[:], negc_d)
        nc.sync.dma_start(eps_t[:], eps_d)
        nc.sync.dma_start(si_t[:], si_d)
        for i in range(3):
            nc.sync.dma_start(sj_t[i][:], sj_d[i])
            nc.sync.dma_start(wj_t[i][:], wj_d[i])

        pj_t = const.tile([P, FS], f16)
        pj4 = pj_t[:].rearrange("p (s k w) -> p s k w", s=NSP, k=K)
        si_b = (
            si_t[:].rearrange("p (s w) -> p s w", s=NSP)
            .unsqueeze(1)
            .broadcast_to([P, K, NSP, WSEG])
        )

        # ---- match: pj[p; s, k, w] = sum_t wj_t * (sj_t == si_s) ----
        # One batched eq/mult over all (t, s) per offset k, then an
        # in-place pairwise tree over t.  Scratch borrows an m rotation.
        em_t = work.tile([P, FS], f16, tag="m", bufs=2, name="em")
        em4 = em_t[:].rearrange("p (t s w) -> p t s w", t=K, s=NSP)
        A = NSP * WSEG  # 864
        for ki, (dh, dw) in enumerate(OFFS):
            sj_b = (
                sj_t[dh + 1][:]
                .rearrange("p (t w) -> p t w", t=K)[:, :, 1 + dw : 97 + dw]
                .unsqueeze(2)
                .broadcast_to([P, K, NSP, WSEG])
            )
            wj_b = (
                wj_t[dh + 1][:]
                .rearrange("p (t w) -> p t w", t=K)[:, :, 1 + dw : 97 + dw]
                .unsqueeze(2)
                .broadcast_to([P, K, NSP, WSEG])
            )
            nc.vector.tensor_tensor(em4, si_b, sj_b, Alu.is_equal)
            nc.vector.tensor_tensor(em4, em4, wj_b, Alu.mult)
            nc.vector.tensor_tensor(
                em_t[:, 0 : 4 * A], em_t[:, 0 : 4 * A],
                em_t[:, 4 * A : 8 * A], Alu.add,
            )
            nc.vector.tensor_tensor(
                em_t[:, 0 : 2 * A], em_t[:, 0 : 2 * A],
                em_t[:, 2 * A : 4 * A], Alu.add,
            )
            nc.vector.tensor_tensor(
                em_t[:, 0:A], em_t[:, 0:A], em_t[:, A : 2 * A], Alu.add
            )
            pjk = pj4[:, :, ki : ki + 1, :]
            nc.vector.tensor_tensor(
                pjk,
                em4[:, 0:1, :, :].transpose([0, 2, 1, 3]),
                em4[:, 8:9, :, :].transpose([0, 2, 1, 3]),
                Alu.add,
            )

        # ---- per-head: pure within-partition DVE chain ----
        for d in range(HD):
            a_t = work.tile([P, FI], f32, tag="a", bufs=4)
            nc.sync.dma_start(a_t[:], a_d[d])
            ae_t = work.tile([P, FI], f16, tag="ae", bufs=4)
            nc.scalar.activation(
                ae_t[:], a_t[:], Act.Exp, bias=negc_t[0:P, :], scale=1.0
            )
            m_t = work.tile([P, FS], bf16, tag="m", bufs=2)
            m4 = m_t[:].rearrange("p (s k w) -> p s k w", s=NSP, k=K)
            ae_b = (
                ae_t[:].rearrange("p (k w) -> p k w", k=K)
                .unsqueeze(1)
                .broadcast_to([P, NSP, K, WSEG])
            )
            nc.vector.tensor_tensor(m4, ae_b, pj4, Alu.mult)
            # den = eps + sum_k m : bf16 pair-tree then f32
            d4_t = work.tile([P, NSP * 4 * WSEG], bf16, tag="d4")
            d44 = d4_t[:].rearrange("p (s k w) -> p s k w", s=NSP, k=4)
            nc.vector.tensor_tensor(
                d44, m4[:, :, 0:4, :], m4[:, :, 4:8, :], Alu.add
            )
            d2_t = work.tile([P, NSP * 2 * WSEG], f32, tag="d2")
            d22 = d2_t[:].rearrange("p (s k w) -> p s k w", s=NSP, k=2)
            nc.vector.tensor_tensor(
                d22, d44[:, :, 0:2, :], d44[:, :, 2:4, :], Alu.add
            )
            den_t = work.tile([P, NSP * WSEG], f32, tag="den")
            den3 = den_t[:].rearrange("p (s w) -> p s w", s=NSP)
            nc.vector.tensor_tensor(
                den3, d22[:, :, 0:1, :].squeeze(2),
                d22[:, :, 1:2, :].squeeze(2), Alu.add,
            )
            # den += eps (per-partition scalar) + m[k=8], in one fused op
            nc.vector.scalar_tensor_tensor(
                den3, den3, eps_t[0:P, :],
                m4[:, :, 8:9, :].squeeze(2),
                Alu.add, Alu.add,
            )
            rec_t = work.tile([P, NSP * WSEG], f32, tag="rec")
            nc.vector.reciprocal_approx_fast(rec_t[:], den_t[:])
            recb_t = work.tile([P, NSP * WSEG], bf16, tag="recb")
            nc.scalar.activation(
                recb_t[:], rec_t[:], Act.Copy, bias=0.0, scale=1.0
            )
            out_t = outp.tile([P, FS], bf16, tag="o", bufs=3)
            o4 = out_t[:].rearrange("p (s k w) -> p s k w", s=NSP, k=K)
            rec_b = (
                recb_t[:].rearrange("p (s w) -> p s w", s=NSP)
                .unsqueeze(2)
                .broadcast_to([P, NSP, K, WSEG])
            )
            nc.vector.tensor_tensor(o4, m4, rec_b, Alu.mult)
            nc.sync.dma_start(out_d[d, :, 0 : FS // 2], out_t[:, 0 : FS // 2])
            nc.sync.dma_start(out_d[d, :, FS // 2 :], out_t[:, FS // 2 :])

    nc.compile()
    return nc


def _get_compiled():
    global _compiled
    if _compiled is None:
        _compiled = _build()
    return _compiled


def _prep_core(attn, sims, sinds, negc, epsv, core):
    b, q = core // 4, core % 4
    h0 = q * ROWS
    cols = np.clip(np.arange(-1, W + 1), 0, W - 1)

    def pad3(x):  # [H, W, 9] -> [3, 96, 9, 98] (dh copies, t-major, w-pad)
        out = np.empty((3, P, K, WSPAN), np.float32)
        for i, dh in enumerate((-1, 0, 1)):
            rows = np.clip(np.arange(h0, h0 + ROWS) + dh, 0, H - 1)
            xp = x[rows][:, cols, :]  # [48, 194, 9]
            segs = [xp[:, WSEG * g : WSEG * g + WSPAN, :] for g in range(2)]
            out[i] = np.concatenate(segs, axis=0).transpose(0, 2, 1)
        return out.reshape(3, P, K * WSPAN)

    sj2 = pad3(sinds[b])
    wj2 = pad3(sims[b]) * 4096.0

    si = sinds[b][h0 : h0 + ROWS]  # [48, 192, 9]
    si = np.concatenate(
        [si[:, WSEG * g : WSEG * (g + 1), :] for g in range(2)], axis=0
    )  # [96, 96, 9]
    si2 = si.transpose(0, 2, 1).reshape(P, NSP * WSEG)

    ap = attn[b][:, h0 : h0 + ROWS]  # [HD, 48, 192, 9]
    ap = np.concatenate(
        [ap[:, :, WSEG * g : WSEG * (g + 1), :] for g in range(2)], axis=1
    )  # [HD, 96, 96, 9]
    a2 = ap.transpose(0, 1, 3, 2).reshape(HD, P, FI)

    return {
        "si2": np.ascontiguousarray(si2).astype(np.float16),
        "sj2": np.ascontiguousarray(sj2).astype(np.float16),
        "wj2": np.ascontiguousarray(wj2).astype(np.float16),
        "a2": np.ascontiguousarray(a2.astype(np.float32)),
        "negc": negc,
        "epsv": epsv,
    }


def kernel(attn, sims, sinds, _trace=False):
    attn = np.asarray(attn)
    sims = np.asarray(sims)
    sinds = np.asarray(sinds).astype(np.float32)

    from concourse import bass_utils

    nc = _get_compiled()

    c = float(np.max(attn))
    # +10 rescales exp into fp16's normal range (cancels in normalization)
    negc = np.full((128, 1), 10.0 - c, dtype=np.float32)
    epsv = np.full((128, 1), EPS_DEV, dtype=np.float32)
    in_maps = [
        _prep_core(attn, sims, sinds, negc, epsv, core) for core in range(NCORES)
    ]
    res = bass_utils.run_bass_kernel_spmd(
        nc, in_maps, core_ids=list(range(NCORES)), trace=_trace
    )
    out = np.empty((B, HD, NSP, H, W, K), dtype=np.float32)
    for core in range(NCORES):
        b, q = core // 4, core % 4
        o = np.asarray(res.results[core]["out"]).astype(np.float32)
        # [d, (half, row), (s, k, w)] -> [d, s, row, (half, w), k]
        o = o.reshape(HD, 2, ROWS, NSP, K, WSEG).transpose(0, 3, 2, 1, 5, 4)
        out[b, :, :, ROWS * q : ROWS * (q + 1)] = o.reshape(
            HD, NSP, ROWS, W, K
        )
    if _trace:
        return out, res
    return out



# revision 2
# speedup vs baseline: 1.2953x; 1.2953x over previous
"""AttnReweight kernel for Trainium2 (8 NeuronCores, SPMD data parallel).

Semantics (matching the reference):
    c = max(attn); a = exp(attn - c)
    pj[b,s,h,w,k] = sum_t sims[b,hj,wj,t] * (sinds[b,hj,wj,t] == sinds[b,h,w,s])
                    where (hj,wj) = clamped 3x3 neighbor k of (h,w)
    m = a[b,d,h,w,k] * pj[b,s,h,w,k]
    out[b,d,s,h,w,k] = m / (1e-10 + sum_k m)

Sharding: core = b*4 + q handles image b, rows [48q, 48q+48), all heads.

On-chip layout (per core): 128 partitions = (wseg 8, row-in-group 16); free
dim = (slot, gw) where gw = g*24 + w fuses the 3 row-groups with the 24-wide
w segment into a contiguous 72-elem inner run.  All 9 (dh,dw) offsets of
sj/wj are pre-shifted (with border clamp) on the host into 9 separate tiles,
so every device op is a clean <=4-dim AP with a 72-wide step-1 inner dim ->
2x DVE packing on every 16-bit op, full 128-lane occupancy.

Precision: fp16 ids/sims/pj (sims x4096), exp rescaled by e^S with S chosen
at runtime so ae stays fp16-normal while 4-term partial sums stay < 65504
(eps scaled to match; cancels in the normalization).  m/out/rec in bf16
(range), den tree fp16 pairs -> fp16 quads -> f32, reciprocal via
exp(-ln(den)) on the otherwise idle Scalar engine (one activation table
set covers Exp and Ln).  Host does the final transpose + f32 cast.
"""

import numpy as np
import ml_dtypes

B, HD, H, W, K, NSP = 2, 8, 192, 192, 9, 9
NCORES = 8
ROWS = 48              # rows per core
NG = 3                 # row-groups per core (16 rows each)
RG = 16                # rows per group (partition sub-index)
NWS = 8                # w segments
WSEG = 24              # w per segment
GW = NG * WSEG         # 72, fused (g, w) inner run
P = NWS * RG           # 128 partitions: p = ws*16 + r
FI = K * GW            # 648  (k, gw) free elements
FS = NSP * K * GW      # 5832 (s, k, gw) free elements
A = NSP * GW           # 648  one t-slice of em
EPS = 1e-10
OFFS = [(dh, dw) for dh in (-1, 0, 1) for dw in (-1, 0, 1)]
BF = ml_dtypes.bfloat16

_compiled = None


def _build():
    from contextlib import ExitStack

    import concourse.bacc as bacc
    import concourse.tile as tile
    from concourse import mybir

    f32 = mybir.dt.float32
    bf16 = mybir.dt.bfloat16
    f16 = mybir.dt.float16
    Alu = mybir.AluOpType
    Act = mybir.ActivationFunctionType

    nc = bacc.Bacc(
        "TRN2",
        target_bir_lowering=False,
        debug=False,
        enable_asserts=True,
        num_devices=NCORES,
    )

    si_d = nc.dram_tensor("si2", [P, NSP * GW], f16, kind="ExternalInput").ap()
    sj_d = [
        nc.dram_tensor(f"sj{i}", [P, K * GW], f16, kind="ExternalInput").ap()
        for i in range(K)
    ]
    wj_d = [
        nc.dram_tensor(f"wj{i}", [P, K * GW], f16, kind="ExternalInput").ap()
        for i in range(K)
    ]
    a_d = nc.dram_tensor("a2", [HD, P, FI], f32, kind="ExternalInput").ap()
    negc_d = nc.dram_tensor("negc", [128, 1], f32, kind="ExternalInput").ap()
    eps_d = nc.dram_tensor("epsv", [128, 1], f32, kind="ExternalInput").ap()
    out_d = nc.dram_tensor("out", [HD, P, FS], bf16, kind="ExternalOutput").ap()

    with tile.TileContext(nc) as tc, ExitStack() as ctx:
        const = ctx.enter_context(tc.tile_pool(name="const", bufs=1))
        work = ctx.enter_context(tc.tile_pool(name="work", bufs=2))
        outp = ctx.enter_context(tc.tile_pool(name="outp", bufs=2))

        negc_t = const.tile([128, 1], f32)
        eps_t = const.tile([128, 1], f32)
        si_t = const.tile([P, NSP * GW], f16)
        sj_t = [const.tile([P, K * GW], f16, name=f"sj{i}") for i in range(K)]
        wj_t = [const.tile([P, K * GW], f16, name=f"wj{i}") for i in range(K)]
        nc.sync.dma_start(negc_t[:], negc_d)
        nc.sync.dma_start(eps_t[:], eps_d)
        nc.sync.dma_start(si_t[:], si_d)
        for i in range(K):
            nc.sync.dma_start(sj_t[i][:], sj_d[i])
            nc.sync.dma_start(wj_t[i][:], wj_d[i])

        # ---- all 8 exps up-front on ScalarE (overlaps the match phase) ----
        ae_t = []
        for d in range(HD):
            a_t = work.tile([P, FI], f32, tag="a", bufs=3)
            nc.sync.dma_start(a_t[:], a_d[d])
            ae = work.tile([P, FI], f16, tag="ae", bufs=HD)
            nc.scalar.activation(
                ae[:], a_t[:], Act.Exp, bias=negc_t[0:P, :], scale=1.0
            )
            ae_t.append(ae)

        pj_t = const.tile([P, FS], f16)
        pj4 = pj_t[:].rearrange("p (s k w) -> p s k w", s=NSP, k=K)
        si_b = (
            si_t[:].rearrange("p (s w) -> p s w", s=NSP)
            .unsqueeze(1)
            .broadcast_to([P, K, NSP, GW])
        )

        # ---- match: pj[p; s, k, gw] = sum_t wj_t * (sj_t == si_s) ----
        for ki in range(K):
            em_t = work.tile([P, FS], f16, tag="em", bufs=2)
            em4 = em_t[:].rearrange("p (t s w) -> p t s w", t=K, s=NSP)
            sj_b = (
                sj_t[ki][:].rearrange("p (t w) -> p t w", t=K)
                .unsqueeze(2)
                .broadcast_to([P, K, NSP, GW])
            )
            wj_b = (
                wj_t[ki][:].rearrange("p (t w) -> p t w", t=K)
                .unsqueeze(2)
                .broadcast_to([P, K, NSP, GW])
            )
            nc.vector.tensor_tensor(em4, si_b, sj_b, Alu.is_equal)
            nc.vector.tensor_tensor(em4, em4, wj_b, Alu.mult)
            nc.vector.tensor_tensor(
                em_t[:, 0 : 4 * A], em_t[:, 0 : 4 * A],
                em_t[:, 4 * A : 8 * A], Alu.add,
            )
            nc.vector.tensor_tensor(
                em_t[:, 0 : 2 * A], em_t[:, 0 : 2 * A],
                em_t[:, 2 * A : 4 * A], Alu.add,
            )
            nc.vector.tensor_tensor(
                em_t[:, 0:A], em_t[:, 0:A], em_t[:, A : 2 * A], Alu.add
            )
            pjk = pj4[:, :, ki : ki + 1, :]
            nc.vector.tensor_tensor(
                pjk,
                em_t[:, 0:A].rearrange("p (s w) -> p s w", s=NSP).unsqueeze(2),
                em_t[:, 8 * A : 9 * A]
                .rearrange("p (s w) -> p s w", s=NSP)
                .unsqueeze(2),
                Alu.add,
            )

        # ---- per-head normalize chain, software-pipelined over heads ----
        prev = None  # (m4, rec_t, d)

        def emit_out(m4p, recp, dp):
            out_t = outp.tile([P, FS], bf16, tag="o", bufs=3)
            o4 = out_t[:].rearrange("p (s k w) -> p s k w", s=NSP, k=K)
            rec_b = (
                recp[:].rearrange("p (s w) -> p s w", s=NSP)
                .unsqueeze(2)
                .broadcast_to([P, NSP, K, GW])
            )
            nc.vector.tensor_tensor(o4, m4p, rec_b, Alu.mult)
            nc.sync.dma_start(out_d[dp, :, 0 : FS // 2], out_t[:, 0 : FS // 2])
            nc.sync.dma_start(out_d[dp, :, FS // 2 :], out_t[:, FS // 2 :])

        for d in range(HD):
            m_t = work.tile([P, FS], bf16, tag="m", bufs=2)
            m4 = m_t[:].rearrange("p (s k w) -> p s k w", s=NSP, k=K)
            ae_b = (
                ae_t[d][:].rearrange("p (k w) -> p k w", k=K)
                .unsqueeze(1)
                .broadcast_to([P, NSP, K, GW])
            )
            nc.vector.tensor_tensor(m4, ae_b, pj4, Alu.mult)
            # den = eps + sum_k m : fp16 pair/quad tree then f32
            t4_t = work.tile([P, NSP * 4 * GW], f16, tag="t4")
            t44 = t4_t[:].rearrange("p (s k w) -> p s k w", s=NSP, k=4)
            nc.vector.tensor_tensor(
                t44, m4[:, :, 0:4, :], m4[:, :, 4:8, :], Alu.add
            )
            t2_t = work.tile([P, NSP * 2 * GW], f16, tag="t2")
            t22 = t2_t[:].rearrange("p (s k w) -> p s k w", s=NSP, k=2)
            nc.vector.tensor_tensor(
                t22, t44[:, :, 0:2, :], t44[:, :, 2:4, :], Alu.add
            )
            den_t = work.tile([P, NSP * GW], f32, tag="den")
            den3 = den_t[:].rearrange("p (s w) -> p s w", s=NSP)
            nc.vector.tensor_tensor(
                den3, t22[:, :, 0:1, :].squeeze(2),
                t22[:, :, 1:2, :].squeeze(2), Alu.add,
            )
            # den += eps (per-partition scalar) + m[k=8], in one fused op
            nc.vector.scalar_tensor_tensor(
                den3, den3, eps_t[0:P, :],
                m4[:, :, 8:9, :].squeeze(2),
                Alu.add, Alu.add,
            )
            # reciprocal on ScalarE: rec = exp(-ln(den)), bf16 cast fused
            lden_t = work.tile([P, NSP * GW], f32, tag="lden")
            nc.scalar.activation(
                lden_t[:], den_t[:], Act.Ln, bias=0.0, scale=1.0
            )
            rec_t = work.tile([P, NSP * GW], bf16, tag="rec")
            nc.scalar.activation(
                rec_t[:], lden_t[:], Act.Exp, bias=0.0, scale=-1.0
            )
            if prev is not None:
                emit_out(*prev)
            prev = (m4, rec_t, d)
        emit_out(*prev)

    nc.compile()
    return nc


def _get_compiled():
    global _compiled
    if _compiled is None:
        _compiled = _build()
    return _compiled


def _prep_core(attn, sims, sinds, negc, epsv, core):
    b, q = core // 4, core % 4
    h0 = q * ROWS

    def to_tiles(x, nslot):
        # x: [48, 192, nslot] -> [P=(ws,r), nslot*GW=(slot, g, w)]
        t = x.reshape(NG, RG, NWS, WSEG, nslot)  # [g, r, ws, w, slot]
        return t.transpose(2, 1, 4, 0, 3).reshape(P, nslot * GW)

    feed = {"negc": negc, "epsv": epsv}
    si = sinds[b, h0 : h0 + ROWS]  # [48, 192, 9]
    feed["si2"] = np.ascontiguousarray(to_tiles(si, NSP)).astype(np.float16)

    wsrc = sims[b] * 4096.0
    for i, (dh, dw) in enumerate(OFFS):
        rs = np.clip(np.arange(h0, h0 + ROWS) + dh, 0, H - 1)
        cs = np.clip(np.arange(W) + dw, 0, W - 1)
        feed[f"sj{i}"] = np.ascontiguousarray(
            to_tiles(sinds[b][rs][:, cs], K)
        ).astype(np.float16)
        feed[f"wj{i}"] = np.ascontiguousarray(
            to_tiles(wsrc[rs][:, cs], K)
        ).astype(np.float16)

    ap = attn[b][:, h0 : h0 + ROWS]  # [HD, 48, 192, 9]
    t = ap.reshape(HD, NG, RG, NWS, WSEG, K)  # [d, g, r, ws, w, k]
    feed["a2"] = np.ascontiguousarray(
        t.transpose(0, 3, 2, 5, 1, 4).reshape(HD, P, FI).astype(np.float32)
    )
    return feed


def kernel(attn, sims, sinds, _trace=False):
    attn = np.asarray(attn)
    sims = np.asarray(sims)
    sinds = np.asarray(sinds).astype(np.float32)

    from concourse import bass_utils

    nc = _get_compiled()

    c = float(np.max(attn))
    span = c - float(np.min(attn))
    # S keeps ae >= fp16 min-normal while 4-term sums stay < fp16 max
    S = min(max(0.9, span - 9.60), 1.29)
    negc = np.full((128, 1), S - c, dtype=np.float32)
    epsv = np.full((128, 1), EPS * np.exp(S) * 4096.0, dtype=np.float32)
    in_maps = [
        _prep_core(attn, sims, sinds, negc, epsv, core) for core in range(NCORES)
    ]
    res = bass_utils.run_bass_kernel_spmd(
        nc, in_maps, core_ids=list(range(NCORES)), trace=_trace
    )
    out = np.empty((B, HD, NSP, H, W, K), dtype=np.float32)
    for core in range(NCORES):
        b, q = core // 4, core % 4
        o = np.asarray(res.results[core]["out"]).astype(np.float32)
        # [d, (ws, r), (s, k, g, w)] -> [d, s, (g, r), (ws, w), k]
        o = o.reshape(HD, NWS, RG, NSP, K, NG, WSEG).transpose(0, 3, 5, 2, 1, 6, 4)
        out[b, :, :, ROWS * q : ROWS * (q + 1)] = o.reshape(
            HD, NSP, ROWS, W, K
        )
    if _trace:
        return out, res
    return out


# revision 4
# speedup vs baseline: 1.3530x; 1.0446x over previous
"""AttnReweight kernel for Trainium2 (8 NeuronCores, SPMD data parallel).

Semantics (matching the reference):
    c = max(attn); a = exp(attn - c)
    pj[b,s,h,w,k] = sum_t sims[b,hj,wj,t] * (sinds[b,hj,wj,t] == sinds[b,h,w,s])
                    where (hj,wj) = clamped 3x3 neighbor k of (h,w)
    m = a[b,d,h,w,k] * pj[b,s,h,w,k]
    out[b,d,s,h,w,k] = m / (1e-10 + sum_k m)

Sharding: core = b*4 + q handles image b, rows [48q, 48q+48), all heads.

On-chip layout (per core): 128 partitions = (wseg 8, row-in-group 16); free
dim = (slot, gw) where gw = g*24 + w fuses the 3 row-groups with the 24-wide
w segment into a contiguous 72-elem inner run.  All 9 (dh,dw) offsets of
sj/wj are pre-shifted (with border clamp) on the host into 9 separate tiles,
so every device op is a clean <=4-dim AP with a 72-wide step-1 inner dim ->
2x DVE packing on every 16-bit op, full 128-lane occupancy.

Precision: fp16 ids/sims/pj (sims x4096), exp rescaled by e^S with S chosen
at runtime so ae stays fp16-normal while 4-term partial sums stay < 65504
(eps scaled to match; cancels in the normalization).  m/out/rec in bf16
(range), den tree fp16 pairs -> fp16 quads -> f32, reciprocal via
exp(-ln(den)) on the otherwise idle Scalar engine (one activation table
set covers Exp and Ln).  Host does the final transpose + f32 cast.
"""

import numpy as np
import ml_dtypes

B, HD, H, W, K, NSP = 2, 8, 192, 192, 9, 9
NCORES = 8
ROWS = 48              # rows per core
NG = 3                 # row-groups per core (16 rows each)
RG = 16                # rows per group (partition sub-index)
NWS = 8                # w segments
WSEG = 24              # w per segment
GW = NG * WSEG         # 72, fused (g, w) inner run
P = NWS * RG           # 128 partitions: p = ws*16 + r
FI = K * GW            # 648  (k, gw) free elements
FS = NSP * K * GW      # 5832 (s, k, gw) free elements
A = NSP * GW           # 648  one t-slice of em
EPS = 1e-10
OFFS = [(dh, dw) for dh in (-1, 0, 1) for dw in (-1, 0, 1)]
BF = ml_dtypes.bfloat16

_compiled = None


def _build():
    from contextlib import ExitStack

    import concourse.bacc as bacc
    import concourse.tile as tile
    from concourse import mybir

    f32 = mybir.dt.float32
    bf16 = mybir.dt.bfloat16
    f16 = mybir.dt.float16
    Alu = mybir.AluOpType
    Act = mybir.ActivationFunctionType

    nc = bacc.Bacc(
        "TRN2",
        target_bir_lowering=False,
        debug=False,
        enable_asserts=True,
        num_devices=NCORES,
    )

    si_d = nc.dram_tensor("si2", [P, NSP * GW], f16, kind="ExternalInput").ap()
    sj_d = [
        nc.dram_tensor(f"sj{i}", [P, K * GW], f16, kind="ExternalInput").ap()
        for i in range(K)
    ]
    wj_d = [
        nc.dram_tensor(f"wj{i}", [P, K * GW], f16, kind="ExternalInput").ap()
        for i in range(K)
    ]
    a_d = nc.dram_tensor("a2", [HD, P, FI], f32, kind="ExternalInput").ap()
    negc_d = nc.dram_tensor("negc", [128, 1], f32, kind="ExternalInput").ap()
    eps_d = nc.dram_tensor("epsv", [128, 1], f32, kind="ExternalInput").ap()
    out_d = nc.dram_tensor("out", [HD, P, FS], bf16, kind="ExternalOutput").ap()

    with tile.TileContext(nc) as tc, ExitStack() as ctx:
        const = ctx.enter_context(tc.tile_pool(name="const", bufs=1))
        work = ctx.enter_context(tc.tile_pool(name="work", bufs=2))
        outp = ctx.enter_context(tc.tile_pool(name="outp", bufs=2))

        negc_t = const.tile([128, 1], f32)
        eps_t = const.tile([128, 1], f32)
        si_t = const.tile([P, NSP * GW], f16)
        sj_t = [const.tile([P, K * GW], f16, name=f"sj{i}") for i in range(K)]
        wj_t = [const.tile([P, K * GW], f16, name=f"wj{i}") for i in range(K)]
        nc.sync.dma_start(negc_t[:], negc_d)
        nc.sync.dma_start(eps_t[:], eps_d)
        # split the critical first tiles across queues to cut the prologue
        HA = NSP * GW // 2
        nc.sync.dma_start(si_t[:, 0:HA], si_d[:, 0:HA])
        nc.sync.dma_start(si_t[:, HA:], si_d[:, HA:])
        nc.sync.dma_start(sj_t[0][:, 0:HA], sj_d[0][:, 0:HA])
        nc.sync.dma_start(sj_t[0][:, HA:], sj_d[0][:, HA:])
        nc.sync.dma_start(wj_t[0][:, 0:HA], wj_d[0][:, 0:HA])
        nc.sync.dma_start(wj_t[0][:, HA:], wj_d[0][:, HA:])
        for i in range(1, K):
            nc.sync.dma_start(sj_t[i][:], sj_d[i])
            nc.sync.dma_start(wj_t[i][:], wj_d[i])

        # ---- all 8 exps up-front on ScalarE (overlaps the match phase) ----
        ae_t = []
        for d in range(HD):
            a_t = work.tile([P, FI], f32, tag="a", bufs=3)
            nc.sync.dma_start(a_t[:], a_d[d])
            ae = work.tile([P, FI], f16, tag="ae", bufs=HD)
            nc.scalar.activation(
                ae[:], a_t[:], Act.Exp, bias=negc_t[0:P, :], scale=1.0
            )
            ae_t.append(ae)

        pj_t = const.tile([P, FS], f16)
        pj4 = pj_t[:].rearrange("p (s k w) -> p s k w", s=NSP, k=K)
        si_b = (
            si_t[:].rearrange("p (s w) -> p s w", s=NSP)
            .unsqueeze(1)
            .broadcast_to([P, K, NSP, GW])
        )

        # ---- match: pj[p; s, k, gw] = sum_t wj_t * (sj_t == si_s) ----
        for ki in range(K):
            em_t = work.tile([P, FS], f16, tag="em", bufs=2)
            em4 = em_t[:].rearrange("p (t s w) -> p t s w", t=K, s=NSP)
            sj_b = (
                sj_t[ki][:].rearrange("p (t w) -> p t w", t=K)
                .unsqueeze(2)
                .broadcast_to([P, K, NSP, GW])
            )
            wj_b = (
                wj_t[ki][:].rearrange("p (t w) -> p t w", t=K)
                .unsqueeze(2)
                .broadcast_to([P, K, NSP, GW])
            )
            nc.vector.tensor_tensor(em4, si_b, sj_b, Alu.is_equal)
            nc.vector.tensor_tensor(em4, em4, wj_b, Alu.mult)
            nc.vector.tensor_tensor(
                em_t[:, 0 : 4 * A], em_t[:, 0 : 4 * A],
                em_t[:, 4 * A : 8 * A], Alu.add,
            )
            nc.vector.tensor_tensor(
                em_t[:, 0 : 2 * A], em_t[:, 0 : 2 * A],
                em_t[:, 2 * A : 4 * A], Alu.add,
            )
            nc.vector.tensor_tensor(
                em_t[:, 0:A], em_t[:, 0:A], em_t[:, A : 2 * A], Alu.add
            )
            pjk = pj4[:, :, ki : ki + 1, :]
            nc.vector.tensor_tensor(
                pjk,
                em_t[:, 0:A].rearrange("p (s w) -> p s w", s=NSP).unsqueeze(2),
                em_t[:, 8 * A : 9 * A]
                .rearrange("p (s w) -> p s w", s=NSP)
                .unsqueeze(2),
                Alu.add,
            )

        # ---- per-head normalize chain, software-pipelined over heads ----
        prev = None  # (m4, rec_t, d)

        def emit_out(m4p, recp, dp):
            out_t = outp.tile([P, FS], bf16, tag="o", bufs=3)
            o4 = out_t[:].rearrange("p (s k w) -> p s k w", s=NSP, k=K)
            rec_b = (
                recp[:].rearrange("p (s w) -> p s w", s=NSP)
                .unsqueeze(2)
                .broadcast_to([P, NSP, K, GW])
            )
            nc.vector.tensor_tensor(o4, m4p, rec_b, Alu.mult)
            nc.sync.dma_start(out_d[dp, :, 0 : FS // 2], out_t[:, 0 : FS // 2])
            nc.sync.dma_start(out_d[dp, :, FS // 2 :], out_t[:, FS // 2 :])

        for d in range(HD):
            m_t = work.tile([P, FS], bf16, tag="m", bufs=2)
            m4 = m_t[:].rearrange("p (s k w) -> p s k w", s=NSP, k=K)
            ae_b = (
                ae_t[d][:].rearrange("p (k w) -> p k w", k=K)
                .unsqueeze(1)
                .broadcast_to([P, NSP, K, GW])
            )
            nc.vector.tensor_tensor(m4, ae_b, pj4, Alu.mult)
            # den = eps + sum_k m : fp16 pair/quad tree then f32
            t4_t = work.tile([P, NSP * 4 * GW], f16, tag="t4")
            t44 = t4_t[:].rearrange("p (s k w) -> p s k w", s=NSP, k=4)
            nc.vector.tensor_tensor(
                t44, m4[:, :, 0:4, :], m4[:, :, 4:8, :], Alu.add
            )
            t2_t = work.tile([P, NSP * 2 * GW], f16, tag="t2")
            t22 = t2_t[:].rearrange("p (s k w) -> p s k w", s=NSP, k=2)
            nc.vector.tensor_tensor(
                t22, t44[:, :, 0:2, :], t44[:, :, 2:4, :], Alu.add
            )
            den_t = work.tile([P, NSP * GW], f32, tag="den")
            den3 = den_t[:].rearrange("p (s w) -> p s w", s=NSP)
            nc.vector.tensor_tensor(
                den3, t22[:, :, 0:1, :].squeeze(2),
                t22[:, :, 1:2, :].squeeze(2), Alu.add,
            )
            # den += eps (per-partition scalar) + m[k=8], in one fused op
            nc.vector.scalar_tensor_tensor(
                den3, den3, eps_t[0:P, :],
                m4[:, :, 8:9, :].squeeze(2),
                Alu.add, Alu.add,
            )
            # reciprocal f32 on DVE, bf16 cast on the idle Scalar engine
            # (Copy is in every activation table set -> no table reloads)
            rcf_t = work.tile([P, NSP * GW], f32, tag="rcf")
            nc.vector.reciprocal_approx_fast(rcf_t[:], den_t[:])
            rec_t = work.tile([P, NSP * GW], bf16, tag="rec")
            nc.scalar.activation(
                rec_t[:], rcf_t[:], Act.Copy, bias=0.0, scale=1.0
            )
            if prev is not None:
                emit_out(*prev)
            prev = (m4, rec_t, d)
        emit_out(*prev)

    nc.compile()
    return nc


def _get_compiled():
    global _compiled
    if _compiled is None:
        _compiled = _build()
    return _compiled


def _prep_core(attn, sims, sinds, negc, epsv, core):
    b, q = core // 4, core % 4
    h0 = q * ROWS

    def to_tiles(x, nslot):
        # x: [48, 192, nslot] -> [P=(ws,r), nslot*GW=(slot, g, w)]
        t = x.reshape(NG, RG, NWS, WSEG, nslot)  # [g, r, ws, w, slot]
        return t.transpose(2, 1, 4, 0, 3).reshape(P, nslot * GW)

    feed = {"negc": negc, "epsv": epsv}
    si = sinds[b, h0 : h0 + ROWS]  # [48, 192, 9]
    feed["si2"] = np.ascontiguousarray(to_tiles(si, NSP)).astype(np.float16)

    wsrc = sims[b] * 4096.0
    for i, (dh, dw) in enumerate(OFFS):
        rs = np.clip(np.arange(h0, h0 + ROWS) + dh, 0, H - 1)
        cs = np.clip(np.arange(W) + dw, 0, W - 1)
        feed[f"sj{i}"] = np.ascontiguousarray(
            to_tiles(sinds[b][rs][:, cs], K)
        ).astype(np.float16)
        feed[f"wj{i}"] = np.ascontiguousarray(
            to_tiles(wsrc[rs][:, cs], K)
        ).astype(np.float16)

    ap = attn[b][:, h0 : h0 + ROWS]  # [HD, 48, 192, 9]
    t = ap.reshape(HD, NG, RG, NWS, WSEG, K)  # [d, g, r, ws, w, k]
    feed["a2"] = np.ascontiguousarray(
        t.transpose(0, 3, 2, 5, 1, 4).reshape(HD, P, FI).astype(np.float32)
    )
    return feed


def kernel(attn, sims, sinds, _trace=False):
    attn = np.asarray(attn)
    sims = np.asarray(sims)
    sinds = np.asarray(sinds).astype(np.float32)

    from concourse import bass_utils

    nc = _get_compiled()

    c = float(np.max(attn))
    span = c - float(np.min(attn))
    # S keeps ae >= fp16 min-normal while 4-term sums stay < fp16 max
    S = min(max(0.9, span - 9.60), 1.29)
    negc = np.full((128, 1), S - c, dtype=np.float32)
    epsv = np.full((128, 1), EPS * np.exp(S) * 4096.0, dtype=np.float32)
    in_maps = [
        _prep_core(attn, sims, sinds, negc, epsv, core) for core in range(NCORES)
    ]
    res = bass_utils.run_bass_kernel_spmd(
        nc, in_maps, core_ids=list(range(NCORES)), trace=_trace
    )
    out = np.empty((B, HD, NSP, H, W, K), dtype=np.float32)
    for core in range(NCORES):
        b, q = core // 4, core % 4
        o = np.asarray(res.results[core]["out"]).astype(np.float32)
        # [d, (ws, r), (s, k, g, w)] -> [d, s, (g, r), (ws, w), k]
        o = o.reshape(HD, NWS, RG, NSP, K, NG, WSEG).transpose(0, 3, 5, 2, 1, 6, 4)
        out[b, :, :, ROWS * q : ROWS * (q + 1)] = o.reshape(
            HD, NSP, ROWS, W, K
        )
    if _trace:
        return out, res
    return out


# revision 9
# speedup vs baseline: 1.3668x; 1.0102x over previous
"""AttnReweight kernel for Trainium2 (8 NeuronCores, SPMD data parallel).

Semantics (matching the reference):
    c = max(attn); a = exp(attn - c)
    pj[b,s,h,w,k] = sum_t sims[b,hj,wj,t] * (sinds[b,hj,wj,t] == sinds[b,h,w,s])
                    where (hj,wj) = clamped 3x3 neighbor k of (h,w)
    m = a[b,d,h,w,k] * pj[b,s,h,w,k]
    out[b,d,s,h,w,k] = m / (1e-10 + sum_k m)

Sharding: core = b*4 + q handles image b, rows [48q, 48q+48), all heads.

On-chip layout (per core): 128 partitions = (wseg 8, row-in-group 16); free
dim = (slot, gw) where gw = g*24 + w fuses the 3 row-groups with the 24-wide
w segment into a contiguous 72-elem inner run.  All 9 (dh,dw) offsets of
sj/wj are pre-shifted (with border clamp) on the host into 9 separate tiles,
so every device op is a clean <=4-dim AP with a 72-wide step-1 inner dim ->
2x DVE packing on every 16-bit op, full 128-lane occupancy.

Precision: fp16 ids/sims/pj (sims x4096), exp rescaled by e^S with S chosen
at runtime so ae stays fp16-normal while 4-term partial sums stay < 65504
(eps scaled to match; cancels in the normalization).  m/out/rec in bf16
(range), den tree fp16 pairs -> fp16 quads -> f32, reciprocal via
exp(-ln(den)) on the otherwise idle Scalar engine (one activation table
set covers Exp and Ln).  Host does the final transpose + f32 cast.
"""

import numpy as np
import ml_dtypes

B, HD, H, W, K, NSP = 2, 8, 192, 192, 9, 9
NCORES = 8
ROWS = 48              # rows per core
NG = 3                 # row-groups per core (16 rows each)
RG = 16                # rows per group (partition sub-index)
NWS = 8                # w segments
WSEG = 24              # w per segment
GW = NG * WSEG         # 72, fused (g, w) inner run
P = NWS * RG           # 128 partitions: p = ws*16 + r
FI = K * GW            # 648  (k, gw) free elements
FS = NSP * K * GW      # 5832 (s, k, gw) free elements
A = NSP * GW           # 648  one t-slice of em
EPS = 1e-10
OFFS = [(dh, dw) for dh in (-1, 0, 1) for dw in (-1, 0, 1)]
BF = ml_dtypes.bfloat16

_compiled = None


def _build():
    from contextlib import ExitStack

    import concourse.bacc as bacc
    import concourse.tile as tile
    from concourse import mybir

    f32 = mybir.dt.float32
    bf16 = mybir.dt.bfloat16
    f16 = mybir.dt.float16
    Alu = mybir.AluOpType
    Act = mybir.ActivationFunctionType

    nc = bacc.Bacc(
        "TRN2",
        target_bir_lowering=False,
        debug=False,
        enable_asserts=True,
        num_devices=NCORES,
    )

    si_d = nc.dram_tensor("si2", [P, NSP * GW], f16, kind="ExternalInput").ap()
    sj_d = [
        nc.dram_tensor(f"sj{i}", [P, K * GW], f16, kind="ExternalInput").ap()
        for i in range(K)
    ]
    wj_d = [
        nc.dram_tensor(f"wj{i}", [P, K * GW], f16, kind="ExternalInput").ap()
        for i in range(K)
    ]
    a_d = nc.dram_tensor("a2", [HD, P, FI], f32, kind="ExternalInput").ap()
    negc_d = nc.dram_tensor("negc", [128, 1], f32, kind="ExternalInput").ap()
    eps_d = nc.dram_tensor("epsv", [128, 1], f32, kind="ExternalInput").ap()
    out_d = nc.dram_tensor("out", [HD, P, FS], bf16, kind="ExternalOutput").ap()

    with tile.TileContext(nc) as tc, ExitStack() as ctx:
        const = ctx.enter_context(tc.tile_pool(name="const", bufs=1))
        work = ctx.enter_context(tc.tile_pool(name="work", bufs=2))
        outp = ctx.enter_context(tc.tile_pool(name="outp", bufs=2))

        negc_t = const.tile([128, 1], f32)
        eps_t = const.tile([128, 1], f32)
        si_t = const.tile([P, NSP * GW], f16)
        sj_t = [const.tile([P, K * GW], f16, name=f"sj{i}") for i in range(K)]
        wj_t = [const.tile([P, K * GW], f16, name=f"wj{i}") for i in range(K)]
        # critical first tiles: issue on the sync queue, split across DMA
        # queues to cut the prologue; everything else issues from ScalarE
        # (also a HWDGE) so instruction issue does not delay the first eq.
        HA = NSP * GW // 2
        nc.sync.dma_start(si_t[:, 0:HA], si_d[:, 0:HA])
        nc.sync.dma_start(si_t[:, HA:], si_d[:, HA:])
        nc.sync.dma_start(sj_t[0][:, 0:HA], sj_d[0][:, 0:HA])
        nc.sync.dma_start(sj_t[0][:, HA:], sj_d[0][:, HA:])
        nc.sync.dma_start(wj_t[0][:, 0:HA], wj_d[0][:, 0:HA])
        nc.sync.dma_start(wj_t[0][:, HA:], wj_d[0][:, HA:])
        nc.scalar.dma_start(negc_t[:], negc_d)
        nc.scalar.dma_start(eps_t[:], eps_d)
        for i in range(1, K):
            nc.scalar.dma_start(sj_t[i][:], sj_d[i])
            nc.scalar.dma_start(wj_t[i][:], wj_d[i])

        # ---- all 8 exps up-front on ScalarE (overlaps the match phase) ----
        ae_t = []
        for d in range(HD):
            a_t = work.tile([P, FI], f32, tag="a", bufs=3)
            nc.scalar.dma_start(a_t[:], a_d[d])
            ae = work.tile([P, FI], f16, tag="ae", bufs=HD)
            nc.scalar.activation(
                ae[:], a_t[:], Act.Exp, bias=negc_t[0:P, :], scale=1.0
            )
            ae_t.append(ae)

        pj_t = const.tile([P, FS], f16)
        pj4 = pj_t[:].rearrange("p (s k w) -> p s k w", s=NSP, k=K)
        si_b = (
            si_t[:].rearrange("p (s w) -> p s w", s=NSP)
            .unsqueeze(1)
            .broadcast_to([P, K, NSP, GW])
        )

        # ---- match: pj[p; s, k, gw] = sum_t wj_t * (sj_t == si_s) ----
        for ki in range(K):
            em_t = work.tile([P, FS], f16, tag="em", bufs=2)
            em4 = em_t[:].rearrange("p (t s w) -> p t s w", t=K, s=NSP)
            sj_b = (
                sj_t[ki][:].rearrange("p (t w) -> p t w", t=K)
                .unsqueeze(2)
                .broadcast_to([P, K, NSP, GW])
            )
            wj_b = (
                wj_t[ki][:].rearrange("p (t w) -> p t w", t=K)
                .unsqueeze(2)
                .broadcast_to([P, K, NSP, GW])
            )
            nc.vector.tensor_tensor(em4, si_b, sj_b, Alu.is_equal)
            nc.vector.tensor_tensor(em4, em4, wj_b, Alu.mult)
            nc.vector.tensor_tensor(
                em_t[:, 0 : 4 * A], em_t[:, 0 : 4 * A],
                em_t[:, 4 * A : 8 * A], Alu.add,
            )
            nc.vector.tensor_tensor(
                em_t[:, 0 : 2 * A], em_t[:, 0 : 2 * A],
                em_t[:, 2 * A : 4 * A], Alu.add,
            )
            nc.vector.tensor_tensor(
                em_t[:, 0:A], em_t[:, 0:A], em_t[:, A : 2 * A], Alu.add
            )
            pjk = pj4[:, :, ki : ki + 1, :]
            nc.vector.tensor_tensor(
                pjk,
                em_t[:, 0:A].rearrange("p (s w) -> p s w", s=NSP).unsqueeze(2),
                em_t[:, 8 * A : 9 * A]
                .rearrange("p (s w) -> p s w", s=NSP)
                .unsqueeze(2),
                Alu.add,
            )

        # ---- per-head normalize chain, software-pipelined over heads ----
        prev = None  # (m4, rec_t, d)

        def emit_out(m4p, recp, dp):
            out_t = outp.tile([P, FS], bf16, tag="o", bufs=3)
            o4 = out_t[:].rearrange("p (s k w) -> p s k w", s=NSP, k=K)
            rec_b = (
                recp[:].rearrange("p (s w) -> p s w", s=NSP)
                .unsqueeze(2)
                .broadcast_to([P, NSP, K, GW])
            )
            nc.vector.tensor_tensor(o4, m4p, rec_b, Alu.mult)
            # last head: 4-way split so the tail drain uses 4 queues
            nsp = 4 if dp == HD - 1 else 2
            step = FS // nsp
            for j in range(nsp):
                nc.sync.dma_start(
                    out_d[dp, :, j * step : (j + 1) * step],
                    out_t[:, j * step : (j + 1) * step],
                )

        for d in range(HD):
            m_t = work.tile([P, FS], bf16, tag="m", bufs=2)
            m4 = m_t[:].rearrange("p (s k w) -> p s k w", s=NSP, k=K)
            ae_b = (
                ae_t[d][:].rearrange("p (k w) -> p k w", k=K)
                .unsqueeze(1)
                .broadcast_to([P, NSP, K, GW])
            )
            nc.vector.tensor_tensor(m4, ae_b, pj4, Alu.mult)
            # den = eps + sum_k m : fp16 pair/quad tree then f32
            t4_t = work.tile([P, NSP * 4 * GW], f16, tag="t4")
            t44 = t4_t[:].rearrange("p (s k w) -> p s k w", s=NSP, k=4)
            nc.vector.tensor_tensor(
                t44, m4[:, :, 0:4, :], m4[:, :, 4:8, :], Alu.add
            )
            t2_t = work.tile([P, NSP * 2 * GW], f16, tag="t2")
            t22 = t2_t[:].rearrange("p (s k w) -> p s k w", s=NSP, k=2)
            nc.vector.tensor_tensor(
                t22, t44[:, :, 0:2, :], t44[:, :, 2:4, :], Alu.add
            )
            t1_t = work.tile([P, NSP * GW], f16, tag="t1")
            t13 = t1_t[:].rearrange("p (s w) -> p s w", s=NSP)
            nc.vector.tensor_tensor(
                t13, t22[:, :, 0:1, :].squeeze(2),
                t22[:, :, 1:2, :].squeeze(2), Alu.add,
            )
            # den = (t1 + eps) + m[k=8], fused, f32 out
            den_t = work.tile([P, NSP * GW], f32, tag="den")
            den3 = den_t[:].rearrange("p (s w) -> p s w", s=NSP)
            nc.vector.scalar_tensor_tensor(
                den3, t13, eps_t[0:P, :],
                m4[:, :, 8:9, :].squeeze(2),
                Alu.add, Alu.add,
            )
            # reciprocal f32 on DVE, bf16 cast on the idle Scalar engine
            # (Copy is in every activation table set -> no table reloads)
            rcf_t = work.tile([P, NSP * GW], f32, tag="rcf")
            nc.vector.reciprocal_approx_fast(rcf_t[:], den_t[:])
            rec_t = work.tile([P, NSP * GW], bf16, tag="rec")
            nc.scalar.activation(
                rec_t[:], rcf_t[:], Act.Copy, bias=0.0, scale=1.0
            )
            if prev is not None:
                emit_out(*prev)
            prev = (m4, rec_t, d)
        emit_out(*prev)

    nc.compile()
    return nc


def _get_compiled():
    global _compiled
    if _compiled is None:
        _compiled = _build()
    return _compiled


def _prep_core(attn, sims, sinds, negc, epsv, core):
    b, q = core // 4, core % 4
    h0 = q * ROWS

    def to_tiles(x, nslot):
        # x: [48, 192, nslot] -> [P=(ws,r), nslot*GW=(slot, g, w)]
        t = x.reshape(NG, RG, NWS, WSEG, nslot)  # [g, r, ws, w, slot]
        return t.transpose(2, 1, 4, 0, 3).reshape(P, nslot * GW)

    feed = {"negc": negc, "epsv": epsv}
    si = sinds[b, h0 : h0 + ROWS]  # [48, 192, 9]
    feed["si2"] = np.ascontiguousarray(to_tiles(si, NSP)).astype(np.float16)

    wsrc = sims[b] * 4096.0
    for i, (dh, dw) in enumerate(OFFS):
        rs = np.clip(np.arange(h0, h0 + ROWS) + dh, 0, H - 1)
        cs = np.clip(np.arange(W) + dw, 0, W - 1)
        feed[f"sj{i}"] = np.ascontiguousarray(
            to_tiles(sinds[b][rs][:, cs], K)
        ).astype(np.float16)
        feed[f"wj{i}"] = np.ascontiguousarray(
            to_tiles(wsrc[rs][:, cs], K)
        ).astype(np.float16)

    ap = attn[b][:, h0 : h0 + ROWS]  # [HD, 48, 192, 9]
    t = ap.reshape(HD, NG, RG, NWS, WSEG, K)  # [d, g, r, ws, w, k]
    feed["a2"] = np.ascontiguousarray(
        t.transpose(0, 3, 2, 5, 1, 4).reshape(HD, P, FI).astype(np.float32)
    )
    return feed


def kernel(attn, sims, sinds, _trace=False):
    attn = np.asarray(attn)
    sims = np.asarray(sims)
    sinds = np.asarray(sinds).astype(np.float32)

    from concourse import bass_utils

    nc = _get_compiled()

    c = float(np.max(attn))
    span = c - float(np.min(attn))
    # S keeps ae >= fp16 min-normal while 8-term sums stay < fp16 max
    S = min(max(0.55, span - 9.70), 0.684)
    negc = np.full((128, 1), S - c, dtype=np.float32)
    epsv = np.full((128, 1), EPS * np.exp(S) * 4096.0, dtype=np.float32)
    in_maps = [
        _prep_core(attn, sims, sinds, negc, epsv, core) for core in range(NCORES)
    ]
    res = bass_utils.run_bass_kernel_spmd(
        nc, in_maps, core_ids=list(range(NCORES)), trace=_trace
    )
    out = np.empty((B, HD, NSP, H, W, K), dtype=np.float32)
    for core in range(NCORES):
        b, q = core // 4, core % 4
        o = np.asarray(res.results[core]["out"]).astype(np.float32)
        # [d, (ws, r), (s, k, g, w)] -> [d, s, (g, r), (ws, w), k]
        o = o.reshape(HD, NWS, RG, NSP, K, NG, WSEG).transpose(0, 3, 5, 2, 1, 6, 4)
        out[b, :, :, ROWS * q : ROWS * (q + 1)] = o.reshape(
            HD, NSP, ROWS, W, K
        )
    if _trace:
        return out, res
    return out
